# revision 1
# baseline (speedup 1.0000x reference)
"""Multi-head attention (qkv proj + 2D RoPE + softmax attention + out proj)
for Trainium2, data-parallel over 8 NeuronCores (one batch element per core).

kernel(**inputs) takes FULL inputs (tensor (8,1024,1024), w_qkv (3072,1024),
w_proj (1024,1024)) and returns the FULL output (8,1024,1024).

Per-core dataflow (one batch element):
  phase 1 (fp32r): qT/kT = Wq^T.T @ xT (feature-on-partition, head-major
           rows), RoPE via cos/sin-permuted tables (2 PSUM-read multiplies +
           4 partition-block-swap DMAs + add); v token-major (bf16),
           scattered into a padded "vbuf" whose constant ones/zeros columns
           fuse the softmax denominator into the attention matmul.
  phase 2: per head pair: transposed scores (fp32r, K=64, row-group
           concurrent), exp on ScalarE -> bf16 p tiles (scale=1/8, no max
           subtraction - max score ~7), then a contiguous burst of bf16
           o^T matmuls with fused row sums; reciprocal + K=1 broadcast
           matmuls normalize into resident oT tiles (fp32r).
  phase 3 (fp32r): out = oT.T @ wpT with streamed weights.
"""
import numpy as np
import ml_dtypes

import concourse.bass as bass
import concourse.bacc as bacc
import concourse.mybir as mybir
import concourse.tile as tile
from concourse import bass_utils

F32 = mybir.dt.float32
F32R = mybir.dt.float32r
BF16 = mybir.dt.bfloat16
AF = mybir.ActivationFunctionType

B, N, C = 8, 1024, 1024
H, HD = 16, 64
HEIGHT = WIDTH = 32
FREQ = 10000.0
NT = N // 128      # 8 row tiles
CT = C // 128      # 8 contraction tiles
PAIRS = H // 2     # 8 head pairs
VSLOT = 193        # [v_e 64 | one | one | z62 | v_o 64]


# ---------------------------------------------------------------- host prep
def _thetas():
    half = HD // 2
    ifreq = (1.0 / (FREQ ** (np.arange(half, dtype=np.float32) / np.float32(half)))).astype(np.float32)
    fh = np.arange(HEIGHT, dtype=np.float32)[:, None] * ifreq[None, :]
    fw = np.arange(WIDTH, dtype=np.float32)[:, None] * ifreq[None, :]
    th = np.broadcast_to(fh[:, None, :], (HEIGHT, WIDTH, half))
    tw = np.broadcast_to(fw[None, :, :], (HEIGHT, WIDTH, half))
    return np.concatenate([th, tw], axis=-1).reshape(N, HD)


def _host_tables():
    theta = _thetas()
    cos = np.cos(theta).astype(np.float32)     # (N, HD)
    sin = np.sin(theta).astype(np.float32)
    cosT2 = np.empty((128, N), np.float32)     # row 64p+d = cos(theta[:, d])
    sinP = np.empty((128, N), np.float32)      # see RoPE permuted-multiply
    for p in range(2):
        for d in range(HD):
            cosT2[64 * p + d] = cos[:, d]
            if d < 32:
                sinP[64 * p + d] = sin[:, d + 32]
            else:
                sinP[64 * p + d] = -sin[:, d - 32]
    # consts: [0:64]=1 (even-bcast mask), [64:320]=0, [320:384]=1 (odd mask)
    consts = np.zeros((128, 384), np.float32)
    consts[:, 0:64] = 1.0
    consts[:, 320:384] = 1.0
    # vbuf constant pattern (bf16): ones at cols 64 and 97 of each slot
    vconst = np.zeros((128, VSLOT * PAIRS), ml_dtypes.bfloat16)
    for s in range(PAIRS):
        vconst[:, VSLOT * s + 64] = 1.0   # even rowsum -> psum row 64
        vconst[:, VSLOT * s + 97] = 1.0   # odd rowsum -> psum row 32
    return cosT2, sinP, consts, vconst


def _host_weights(w_qkv, w_proj):
    w3 = np.asarray(w_qkv, np.float32).reshape(H, 3 * HD, C)
    wqT = np.ascontiguousarray(w3[:, 0:HD].reshape(H * HD, C).T)
    wkT = np.ascontiguousarray(w3[:, HD:2 * HD].reshape(H * HD, C).T)
    wvT = np.ascontiguousarray(w3[:, 2 * HD:3 * HD].reshape(H * HD, C).T)
    wpT = np.ascontiguousarray(np.asarray(w_proj, np.float32).T)
    return wqT, wkT, wvT, wpT


# ---------------------------------------------------------------- bass build
def build_kernel(nc):
    xT_d = nc.dram_tensor("xT", [C, N], F32, kind="ExternalInput").ap()
    wqT_d = nc.dram_tensor("wqT", [C, C], F32, kind="ExternalInput").ap()
    wkT_d = nc.dram_tensor("wkT", [C, C], F32, kind="ExternalInput").ap()
    wvT_d = nc.dram_tensor("wvT", [C, C], F32, kind="ExternalInput").ap()
    wpT_d = nc.dram_tensor("wpT", [C, C], F32, kind="ExternalInput").ap()
    cos_d = nc.dram_tensor("cosT2", [128, N], F32, kind="ExternalInput").ap()
    sinp_d = nc.dram_tensor("sinP", [128, N], F32, kind="ExternalInput").ap()
    con_d = nc.dram_tensor("consts", [128, 384], F32, kind="ExternalInput").ap()
    vcon_d = nc.dram_tensor("vconst", [128, VSLOT * PAIRS], BF16,
                            kind="ExternalInput").ap()
    out_d = nc.dram_tensor("out", [N, C], F32, kind="ExternalOutput").ap()

    with tile.TileContext(nc) as tc:
        _body(tc, xT_d, wqT_d, wkT_d, wvT_d, wpT_d, cos_d, sinp_d, con_d,
              vcon_d, out_d)
    return nc


def _body(tc, xT_d, wqT_d, wkT_d, wvT_d, wpT_d, cos_d, sinp_d, con_d,
          vcon_d, out_d):
    nc = tc.nc

    with tc.tile_pool(name="persist", bufs=1) as persist, \
         tc.tile_pool(name="cpool", bufs=1) as cpool:
        qR = [persist.tile([128, N], F32R, tag=f"qR{t}", name=f"qR{t}")
              for t in range(PAIRS)]
        kR = [persist.tile([128, N], F32R, tag=f"kR{t}", name=f"kR{t}")
              for t in range(PAIRS)]
        vbuf = [persist.tile([128, VSLOT * PAIRS], BF16, tag=f"vb{tn}",
                             name=f"vb{tn}") for tn in range(NT)]
        oT = [persist.tile([128, N], F32R, tag=f"oT{t}", name=f"oT{t}")
              for t in range(PAIRS)]
        csb = cpool.tile([128, 384], F32R, tag="csb")
        nc.sync.dma_start(csb[:], con_d[:].bitcast(F32R))

        # PE warm-up: fill the initial input-DMA window with dummy matmuls
        # on the constants tile so the HAM un-throttles before phase 1.
        with tc.tile_pool(name="warm", bufs=1, space="PSUM") as wpsum:
            wt = wpsum.tile([128, 384], F32, tag="warm", name="warm")
            for _ in range(64):
                nc.tensor.matmul(wt[:], csb[:, 0:128], csb[:],
                                 start=True, stop=True)

        # -------------------------------------------- phase 1
        with tc.tile_pool(name="tables", bufs=1) as tables, \
             tc.tile_pool(name="xbuf", bufs=1) as xpool, \
             tc.tile_pool(name="wstream", bufs=4) as wpool, \
             tc.tile_pool(name="rope", bufs=3) as rpool, \
             tc.tile_pool(name="pj", bufs=8, space="PSUM") as ppj:

            xT = [xpool.tile([128, N], F32R, tag=f"xT{kc}", name=f"xT{kc}")
                  for kc in range(CT)]
            for kc in range(CT):
                nc.sync.dma_start(
                    xT[kc][:], xT_d[128 * kc:128 * (kc + 1), :].bitcast(F32R))
            cos_sb = tables.tile([128, N], F32R, tag="cos")
            nc.sync.dma_start(cos_sb[:], cos_d[:].bitcast(F32R))
            sinp_sb = tables.tile([128, N], F32R, tag="sinp")
            nc.sync.dma_start(sinp_sb[:], sinp_d[:].bitcast(F32R))

            def proj_rope(w_d, dest):
                for j in range(2):
                    sl = slice(512 * j, 512 * (j + 1))
                    ps = [ppj.tile([128, 512], F32, tag="pj", name="pj")
                          for _ in range(PAIRS)]
                    for kc in range(CT):
                        w = wpool.tile([128, C], F32R, tag="w")
                        nc.sync.dma_start(
                            w[:], w_d[128 * kc:128 * (kc + 1), :].bitcast(F32R))
                        for t in range(PAIRS):
                            nc.tensor.matmul(
                                ps[t][:], w[:, 128 * t:128 * (t + 1)],
                                xT[kc][:, sl],
                                start=(kc == 0), stop=(kc == CT - 1))
                    for t in range(PAIRS):
                        qraw = rpool.tile([128, 512], F32R, tag="qraw",
                                          name="qraw")
                        nc.scalar.copy(qraw[:], ps[t][:])
                        u = rpool.tile([128, 512], F32R, tag="u")
                        up = rpool.tile([128, 512], F32R, tag="up")
                        nc.vector.tensor_mul(u[:], qraw[:], sinp_sb[:, sl])
                        nc.vector.tensor_mul(dest[t][:, sl], qraw[:],
                                             cos_sb[:, sl])
                        for blk in range(4):
                            s = 32 * ((blk // 2) * 2 + 1 - (blk % 2))
                            d = 32 * blk
                            nc.sync.dma_start(up[d:d + 32, :], u[s:s + 32, :])
                        nc.vector.tensor_add(dest[t][:, sl], dest[t][:, sl],
                                             up[:])

            proj_rope(wqT_d, qR)
            proj_rope(wkT_d, kR)

            # vbuf constant pattern (one DMA per row tile), then v scatter
            for tn in range(NT):
                nc.sync.dma_start(vbuf[tn][:], vcon_d[:])
            for jc in range(2):
                ps = [ppj.tile([128, 512], F32, tag="pj", name="pj")
                      for _ in range(NT)]
                for kc in range(CT):
                    w = wpool.tile([128, C], F32R, tag="w")
                    nc.sync.dma_start(
                        w[:], wvT_d[128 * kc:128 * (kc + 1), :].bitcast(F32R))
                    for tn in range(NT):
                        nc.tensor.matmul(
                            ps[tn][:], xT[kc][:, 128 * tn:128 * (tn + 1)],
                            w[:, 512 * jc:512 * (jc + 1)],
                            start=(kc == 0), stop=(kc == CT - 1))
                for tn in range(NT):
                    vsrc = ps[tn][:].rearrange("p (h c) -> p h c", h=8, c=64)
                    vb = vbuf[tn][:].rearrange("p (s c) -> p s c",
                                               s=PAIRS, c=VSLOT)
                    pv = slice(4 * jc, 4 * jc + 4)
                    nc.any.tensor_copy(vb[:, pv, 0:64], vsrc[:, 0::2])
                    nc.any.tensor_copy(vb[:, pv, 129:193], vsrc[:, 1::2])

        # -------------------------------------------- phase 2
        with tc.tile_pool(name="psc", bufs=2, space="PSUM") as psc, \
             tc.tile_pool(name="poe", bufs=2, space="PSUM") as poe, \
             tc.tile_pool(name="poo", bufs=2, space="PSUM") as poo, \
             tc.tile_pool(name="pp", bufs=14) as ppool, \
             tc.tile_pool(name="ns", bufs=2) as nspool:

            for t in range(PAIRS):
                # scores + exp for the whole pair (p tiles in bf16)
                pts = []
                for i in range(NT):
                    p_e = ppool.tile([128, N], BF16, tag="pe", name="pe")
                    p_o = ppool.tile([128, N], BF16, tag="po", name="po")
                    for par, p_sb in ((0, p_e), (1, p_o)):
                        sc = psc.tile([128, N], F32, tag="sc", name="sc")
                        pr = slice(64 * par, 64 * par + 64)
                        for j in range(2):
                            nc.tensor.matmul(
                                sc[:, 512 * j:512 * (j + 1)],
                                kR[t][pr, 128 * i:128 * (i + 1)],
                                qR[t][pr, 512 * j:512 * (j + 1)],
                                start=True, stop=True)
                        nc.scalar.activation(p_sb[:], sc[:], AF.Exp,
                                             scale=0.125)
                    pts.append((p_e, p_o))
                # contiguous o-matmul bursts per nq chunk, then normalize
                for j in range(2):
                    sl = slice(512 * j, 512 * (j + 1))
                    o_e = poe.tile([65, 512], F32, tag="oe", name="oe")
                    o_o = poo.tile([128, 512], F32, tag="oo", name="oo")
                    for i in range(NT):
                        vb = vbuf[i][:]
                        nc.tensor.matmul(
                            o_e[:], vb[:, VSLOT * t:VSLOT * t + 65],
                            pts[i][0][:, sl],
                            start=(i == 0), stop=(i == NT - 1))
                        nc.tensor.matmul(
                            o_o[:], vb[:, VSLOT * t + 65:VSLOT * (t + 1)],
                            pts[i][1][:, sl],
                            start=(i == 0), stop=(i == NT - 1))
                    # normalize: evacuate denominator rows (ACT), one
                    # reciprocal over rows [0:65] (rows 1:63 unused garbage),
                    # K=1 row-group broadcasts, scale
                    rec_e = nspool.tile([128, 512], F32R, tag="rece", name="rece")
                    rec_o = nspool.tile([128, 512], F32R, tag="reco", name="reco")
                    with nc.allow_low_precision(reason="f32r recip feeds bcast"):
                        nc.vector.reciprocal(rec_e[64:65, :], o_e[64:65, :])
                        nc.vector.reciprocal(rec_o[32:33, :], o_o[32:33, :])
                    s_ps = psc.tile([128, N], F32, tag="sc", name="sps")
                    nc.tensor.matmul(s_ps[:, 0:512], csb[64:65, 0:128],
                                     rec_e[64:65, :], start=True, stop=False,
                                     tile_position=(64, 0))
                    nc.tensor.matmul(s_ps[:, 0:512], csb[32:33, 256:384],
                                     rec_o[32:33, :], start=False, stop=True,
                                     tile_position=(32, 0))
                    s_sb = nspool.tile([128, 512], F32, tag="ssb")
                    nc.any.tensor_copy(s_sb[:], s_ps[:, 0:512])
                    nc.vector.tensor_mul(oT[t][0:64, sl], o_e[0:64, :],
                                         s_sb[0:64, :])
                    nc.vector.tensor_mul(oT[t][64:128, sl], o_o[64:128, :],
                                         s_sb[64:128, :])

        # -------------------------------------------- phase 3
        with tc.tile_pool(name="wp3", bufs=8) as wpool3, \
             tc.tile_pool(name="ob", bufs=3) as opool, \
             tc.tile_pool(name="po3", bufs=8, space="PSUM") as ppo:
            wp = []
            for ct in range(CT):
                w = wpool3.tile([128, C], F32R, tag="wp", name="wp")
                nc.sync.dma_start(
                    w[:], wpT_d[128 * ct:128 * (ct + 1), :].bitcast(F32R))
                wp.append(w)
            for jc in range(2):
                ps = [ppo.tile([128, 512], F32, tag="po", name="po")
                      for _ in range(NT)]
                for ct in range(CT):
                    for tn in range(NT):
                        nc.tensor.matmul(ps[tn][:],
                                         oT[ct][:, 128 * tn:128 * (tn + 1)],
                                         wp[ct][:, 512 * jc:512 * (jc + 1)],
                                         start=(ct == 0), stop=(ct == CT - 1))
                for tn in range(NT):
                    ob = opool.tile([128, 512], F32, tag="ob")
                    nc.any.tensor_copy(ob[:], ps[tn][:])
                    nc.sync.dma_start(
                        out_d[128 * tn:128 * (tn + 1), 512 * jc:512 * (jc + 1)],
                        ob[:])


# ---------------------------------------------------------------- entry
_CACHE = {}


def _get_nc():
    if "nc" not in _CACHE:
        nc = bacc.Bacc("TRN2", target_bir_lowering=False, debug=False,
                       num_devices=B)
        build_kernel(nc)
        nc.compile()
        _CACHE["nc"] = nc
    return _CACHE["nc"]


def make_in_maps(tensor, w_qkv, w_proj):
    tensor = np.asarray(tensor, np.float32)
    wqT, wkT, wvT, wpT = _host_weights(w_qkv, w_proj)
    cosT2, sinP, consts, vconst = _host_tables()
    shared = {"wqT": wqT, "wkT": wkT, "wvT": wvT, "wpT": wpT,
              "cosT2": cosT2, "sinP": sinP, "consts": consts,
              "vconst": vconst}
    in_maps = []
    for i in range(B):
        m = dict(shared)
        m["xT"] = np.ascontiguousarray(tensor[i].T)
        in_maps.append(m)
    return in_maps


def run(tensor, w_qkv, w_proj, trace=False):
    in_maps = make_in_maps(tensor, w_qkv, w_proj)
    nc = _get_nc()
    res = bass_utils.run_bass_kernel_spmd(nc, in_maps, core_ids=list(range(B)),
                                          trace=trace)
    out = np.stack([res.results[i]["out"] for i in range(B)])
    return out, res


def kernel(tensor, w_qkv, w_proj):
    out, _ = run(tensor, w_qkv, w_proj, trace=False)
    return out.astype(np.float32)



# revision 11
# speedup vs baseline: 1.5621x; 1.5621x over previous
"""Multi-head attention (qkv proj + 2D RoPE + softmax attention + out proj)
for Trainium2, data-parallel over 8 NeuronCores (one batch element per core).

kernel(**inputs) takes FULL inputs (tensor (8,1024,1024), w_qkv (3072,1024),
w_proj (1024,1024)) and returns the FULL output (8,1024,1024).

v2 -- all-bf16 interleaved pipeline:
  - every matmul operand bf16 (host-cast): FWL weight loads, half DMA/SBUF
    traffic; PSUM accumulation stays fp32 (except scores, see below).
  - weights SBUF-resident (one DMA pass).
  - scores land in bf16 PSUM ([128,1024] = 1 bank), so the proj pool
    (4 banks) + scores (2) + attnV o-psums (1+1) coexist and projection,
    scores+exp, and attnV interleave per (pair, token-half) unit -- the PE
    never idles long enough for the HAM to re-throttle.
  - softmax denominators ride the attnV matmuls via ones columns in vbuf;
    1/den via reciprocal_approx_fast on the psum rows, broadcast across
    partitions with GpSimd partition_broadcast (SBUF->SBUF, no PSUM use).
  - RoPE: ACT evacuates proj psum -> DVE *cos -> GpSimd *sin/add; the
    32-row block swap stays on SBUF-SBUF DMA (written back into the dead
    qraw tile).
  - oT aliases qR (qR[t] is dead once scores(t) issued).
  - out-proj streams in 2-bank chunks through a pool that reuses the
    projection psum banks; wp/out-staging SBUF comes from the closed
    projection pool.
"""
import numpy as np
import ml_dtypes

import concourse.bass as bass
import concourse.bacc as bacc
import concourse.mybir as mybir
import concourse.tile as tile
from concourse import bass_utils

F32 = mybir.dt.float32
F32R = mybir.dt.float32r
BF16 = mybir.dt.bfloat16
AF = mybir.ActivationFunctionType

B, N, C = 8, 1024, 1024
H, HD = 16, 64
HEIGHT = WIDTH = 32
FREQ = 10000.0
NT = N // 128      # 8 token tiles
CT = C // 128      # 8 contraction tiles
PAIRS = H // 2     # 8 head pairs
VSLOT = 193        # [v_e 64 | one | one | z62 | v_o 64]


# ---------------------------------------------------------------- host prep
def _thetas():
    half = HD // 2
    ifreq = (1.0 / (FREQ ** (np.arange(half, dtype=np.float32) / np.float32(half)))).astype(np.float32)
    fh = np.arange(HEIGHT, dtype=np.float32)[:, None] * ifreq[None, :]
    fw = np.arange(WIDTH, dtype=np.float32)[:, None] * ifreq[None, :]
    th = np.broadcast_to(fh[:, None, :], (HEIGHT, WIDTH, half))
    tw = np.broadcast_to(fw[None, :, :], (HEIGHT, WIDTH, half))
    return np.concatenate([th, tw], axis=-1).reshape(N, HD)


def _host_tables():
    theta = _thetas()
    cos = np.cos(theta).astype(np.float32)     # (N, HD)
    sin = np.sin(theta).astype(np.float32)
    cosT2 = np.empty((128, N), np.float32)     # row 64p+d = cos(theta[:, d])
    sinP = np.empty((128, N), np.float32)      # permuted-sign sin
    for p in range(2):
        for d in range(HD):
            cosT2[64 * p + d] = cos[:, d]
            if d < 32:
                sinP[64 * p + d] = sin[:, d + 32]
            else:
                sinP[64 * p + d] = -sin[:, d - 32]
    # vbuf constant pattern (bf16): ones at cols 64 and 97 of each slot
    vconst = np.zeros((128, VSLOT * PAIRS), ml_dtypes.bfloat16)
    for s in range(PAIRS):
        vconst[:, VSLOT * s + 64] = 1.0   # even rowsum -> psum row 64
        vconst[:, VSLOT * s + 97] = 1.0   # odd rowsum -> psum row 32
    # consts: [0:64]=1 (even-bcast mask), [64:320]=0, [320:384]=1 (odd mask)
    consts = np.zeros((128, 384), np.float32)
    consts[:, 0:64] = 1.0
    consts[:, 320:384] = 1.0
    bf = ml_dtypes.bfloat16
    return cosT2.astype(bf), sinP.astype(bf), vconst, consts


def _host_weights(w_qkv, w_proj):
    w3 = np.asarray(w_qkv, np.float32).reshape(H, 3 * HD, C)
    wqT = np.ascontiguousarray(w3[:, 0:HD].reshape(H * HD, C).T)
    wkT = np.ascontiguousarray(w3[:, HD:2 * HD].reshape(H * HD, C).T)
    wvT = np.ascontiguousarray(w3[:, 2 * HD:3 * HD].reshape(H * HD, C).T)
    wpT = np.ascontiguousarray(np.asarray(w_proj, np.float32).T)
    bf = ml_dtypes.bfloat16
    return wqT.astype(bf), wkT.astype(bf), wvT.astype(bf), wpT.astype(bf)


# ---------------------------------------------------------------- bass build
def build_kernel(nc):
    xT_d = nc.dram_tensor("xT", [C, N], BF16, kind="ExternalInput").ap()
    wqT_d = nc.dram_tensor("wqT", [C, C], BF16, kind="ExternalInput").ap()
    wkT_d = nc.dram_tensor("wkT", [C, C], BF16, kind="ExternalInput").ap()
    wvT_d = nc.dram_tensor("wvT", [C, C], BF16, kind="ExternalInput").ap()
    wpT_d = nc.dram_tensor("wpT", [C, C], BF16, kind="ExternalInput").ap()
    cos_d = nc.dram_tensor("cosT2", [128, N], BF16, kind="ExternalInput").ap()
    sinp_d = nc.dram_tensor("sinP", [128, N], BF16, kind="ExternalInput").ap()
    vcon_d = nc.dram_tensor("vconst", [128, VSLOT * PAIRS], BF16,
                            kind="ExternalInput").ap()
    con_d = nc.dram_tensor("consts", [128, 384], F32, kind="ExternalInput").ap()
    out_d = nc.dram_tensor("out", [N, C], F32, kind="ExternalOutput").ap()

    with tile.TileContext(nc) as tc:
        _body(tc, xT_d, wqT_d, wkT_d, wvT_d, wpT_d, cos_d, sinp_d, vcon_d,
              con_d, out_d)
    return nc


def _body(tc, xT_d, wqT_d, wkT_d, wvT_d, wpT_d, cos_d, sinp_d, vcon_d,
          con_d, out_d):
    nc = tc.nc

    with tc.tile_pool(name="persist", bufs=1) as persist, \
         tc.tile_pool(name="pp", bufs=4) as ppool, \
         tc.tile_pool(name="rec", bufs=2) as recpool, \
         tc.tile_pool(name="rbc", bufs=1) as rbcpool, \
         tc.tile_pool(name="rp", bufs=2) as rpool, \
         tc.tile_pool(name="psc", bufs=2, space="PSUM") as psc:

        # ---------------- persistent tiles + input DMAs
        vbuf = [persist.tile([128, VSLOT * PAIRS], BF16, tag=f"vb{tn}",
                             name=f"vb{tn}") for tn in range(NT)]
        for tn in range(NT):
            nc.sync.dma_start(vbuf[tn][:], vcon_d[:])
        csb = persist.tile([128, 384], F32R, tag="csb")
        nc.sync.dma_start(csb[:], con_d[:].bitcast(F32R))
        qR = [persist.tile([128, N], BF16, tag=f"qR{t}", name=f"qR{t}")
              for t in range(PAIRS)]
        kR = [persist.tile([128, N], BF16, tag=f"kR{t}", name=f"kR{t}")
              for t in range(PAIRS)]
        oT = qR   # qR[t] is dead after scores(t); attnV(t) reuses it as oT

        # projection-lifetime pool (closed before out-proj; wp/ob reuse it)
        prx_cm = tc.tile_pool(name="prx", bufs=1)
        prx = prx_cm.__enter__()
        xT = [prx.tile([128, N], BF16, tag=f"xT{kc}", name=f"xT{kc}")
              for kc in range(CT)]
        for kc in range(CT):
            nc.sync.dma_start(xT[kc][:], xT_d[128 * kc:128 * (kc + 1), :])
        wq = [prx.tile([128, C], BF16, tag=f"wq{kc}", name=f"wq{kc}")
              for kc in range(CT)]
        for kc in range(CT):
            nc.sync.dma_start(wq[kc][:], wqT_d[128 * kc:128 * (kc + 1), :])
        wk = [prx.tile([128, C], BF16, tag=f"wk{kc}", name=f"wk{kc}")
              for kc in range(CT)]
        for kc in range(CT):
            nc.sync.dma_start(wk[kc][:], wkT_d[128 * kc:128 * (kc + 1), :])
        cos_sb = prx.tile([128, N], BF16, tag="cos")
        nc.sync.dma_start(cos_sb[:], cos_d[:])
        sinp_sb = prx.tile([128, N], BF16, tag="sinp")
        nc.sync.dma_start(sinp_sb[:], sinp_d[:])
        wv = [prx.tile([128, C], BF16, tag=f"wv{kc}", name=f"wv{kc}")
              for kc in range(CT)]
        for kc in range(CT):
            nc.sync.dma_start(wv[kc][:], wvT_d[128 * kc:128 * (kc + 1), :])

        def rope_tile(tag):
            return rpool.tile([128, 512], F32, tag=tag, name=tag)

        # probability tiles: tag (par, ih), ring of 4 units (= 2 pairs)
        pts = {}   # (t, j, par, i) -> (tile, col offset)
        attnp = {}

        def scores(t, j):
            for par in range(2):
                pr = slice(64 * par, 64 * par + 64)
                for ih in range(4):
                    sc = psc.tile([128, N], F32, tag="sc", name="sc")
                    for il in range(2):
                        i = 2 * ih + il
                        nc.tensor.matmul(
                            sc[:, 512 * il:512 * il + 512],
                            kR[t][pr, 128 * i:128 * (i + 1)],
                            qR[t][pr, 512 * j:512 * (j + 1)],
                            start=True, stop=True)
                    p_sb = ppool.tile([128, N], BF16, tag=f"p{par}{ih}",
                                      name=f"p{par}{ih}")
                    nc.scalar.activation(p_sb[:], sc[:], AF.Exp, scale=0.125)
                    for il in range(2):
                        pts[(t, j, par, 2 * ih + il)] = (p_sb, 512 * il)

        def attn_v(t, j):
            sl = slice(512 * j, 512 * (j + 1))
            o_e = attnp["poe"].tile([65, 512], F32, tag="oe", name="oe")
            o_o = attnp["poo"].tile([128, 512], F32, tag="oo", name="oo")
            for i in range(NT):
                vb = vbuf[i][:]
                pe_t, pe_off = pts[(t, j, 0, i)]
                po_t, po_off = pts[(t, j, 1, i)]
                nc.tensor.matmul(o_e[:], vb[:, VSLOT * t:VSLOT * t + 65],
                                 pe_t[:, pe_off:pe_off + 512],
                                 start=(i == 0), stop=(i == NT - 1))
                nc.tensor.matmul(o_o[:],
                                 vb[:, VSLOT * t + 65:VSLOT * (t + 1)],
                                 po_t[:, po_off:po_off + 512],
                                 start=(i == 0), stop=(i == NT - 1))
            from concourse.dve_ops import (RECIP_APPROX_FAST_CONSTS,
                                           RECIPROCAL_APPROX_FAST)
            _rc = RECIP_APPROX_FAST_CONSTS
            recE = recpool.tile([128, 512], F32R, tag="rec")
            recO = recpool.tile([128, 512], F32R, tag="rec")
            with nc.allow_low_precision(reason="f32r recip feeds bcast"):
                nc.vector._custom_dve(RECIPROCAL_APPROX_FAST,
                                      out=recE[0:65, :], in0=o_e[0:65, :],
                                      s0=_rc["s0"], s1=_rc["s1"],
                                      imm2=_rc["imm2"])
                nc.vector._custom_dve(RECIPROCAL_APPROX_FAST,
                                      out=recO[0:128, :], in0=o_o[0:128, :],
                                      s0=_rc["s0"], s1=_rc["s1"],
                                      imm2=_rc["imm2"])
            s_ps = attnp["sps"].tile([128, 512], F32, tag="sps", name="sps")
            nc.tensor.matmul(s_ps[:], csb[64:65, 0:128], recE[64:65, :],
                             start=True, stop=False, tile_position=(64, 0))
            nc.tensor.matmul(s_ps[:], csb[32:33, 256:384], recO[32:33, :],
                             start=False, stop=True, tile_position=(32, 0))
            rbc = rbcpool.tile([128, 512], F32, tag="rbc")
            nc.vector.tensor_copy(rbc[:], s_ps[:])
            with nc.allow_low_precision(reason="bf16 attention output"):
                nc.vector.tensor_mul(oT[t][0:64, sl], o_e[0:64, :],
                                     rbc[0:64, :])
                nc.vector.tensor_mul(oT[t][64:128, sl], o_o[64:128, :],
                                     rbc[64:128, :])

        pj_cm = tc.tile_pool(name="pj", bufs=4, space="PSUM")
        ppj = pj_cm.__enter__()

        # PE warm-up on the vconst tile while input DMAs land.
        for w in range(12):
            wt = ppj.tile([128, 512], F32, tag="pj", name="warm")
            for _ in range(4):
                nc.tensor.matmul(wt[:, 0:128], vbuf[0][:, 0:128],
                                 vbuf[0][:, 0:128], start=True, stop=True)

        def proj_rope(w_tiles, dest, t0):
            ps = [ppj.tile([128, 512], F32, tag="pj", name="pq")
                  for _ in range(4)]
            for kc in range(CT):
                for tl in range(2):
                    t = t0 + tl
                    for j in range(2):
                        nc.tensor.matmul(
                            ps[2 * tl + j][:],
                            w_tiles[kc][:, 128 * t:128 * (t + 1)],
                            xT[kc][:, 512 * j:512 * (j + 1)],
                            start=(kc == 0), stop=(kc == CT - 1))
            for tl in range(2):
                t = t0 + tl
                for j in range(2):
                    sl = slice(512 * j, 512 * (j + 1))
                    qraw = rope_tile("qraw")
                    nc.scalar.copy(qraw[:], ps[2 * tl + j][:])
                    t1 = rope_tile("t1")
                    nc.vector.tensor_mul(t1[:], qraw[:], cos_sb[:, sl])
                    u = rpool.tile([128, 512], BF16, tag="u", name="u")
                    nc.gpsimd.tensor_mul(u[:], qraw[:], sinp_sb[:, sl])
                    usw = rpool.tile([128, 512], BF16, tag="usw", name="usw")
                    for blk in range(4):
                        s = 32 * ((blk // 2) * 2 + 1 - (blk % 2))
                        d = 32 * blk
                        nc.sync.dma_start(usw[d:d + 32, :], u[s:s + 32, :])
                    with nc.allow_low_precision(reason="bf16 q/k"):
                        nc.gpsimd.tensor_add(dest[t][:, sl], t1[:], usw[:])

        def v_proj():
            for jc in range(2):
                for tb in range(2):
                    ps = [ppj.tile([128, 512], F32, tag="pj", name="pv")
                          for _ in range(4)]
                    for kc in range(CT):
                        for x in range(4):
                            tn = 4 * tb + x
                            nc.tensor.matmul(
                                ps[x][:],
                                xT[kc][:, 128 * tn:128 * (tn + 1)],
                                wv[kc][:, 512 * jc:512 * (jc + 1)],
                                start=(kc == 0), stop=(kc == CT - 1))
                    for x in range(4):
                        tn = 4 * tb + x
                        vsrc = ps[x][:].rearrange("p (h c) -> p h c",
                                                  h=8, c=64)
                        vb = vbuf[tn][:].rearrange("p (s c) -> p s c",
                                                   s=PAIRS, c=VSLOT)
                        pv = slice(4 * jc, 4 * jc + 4)
                        nc.vector.tensor_copy(vb[:, pv, 0:64], vsrc[:, 0::2])
                        nc.vector.tensor_copy(vb[:, pv, 129:193],
                                              vsrc[:, 1::2])

        # ---------------- schedule: proj (+ first scores), then attention
        v_proj()
        proj_rope(wq, qR, 0)
        proj_rope(wk, kR, 0)
        scores(0, 0)
        scores(0, 1)
        proj_rope(wq, qR, 2)
        proj_rope(wk, kR, 2)
        scores(1, 0)
        scores(1, 1)
        proj_rope(wq, qR, 4)
        proj_rope(wk, kR, 4)
        proj_rope(wq, qR, 6)
        proj_rope(wk, kR, 6)

        pj_cm.__exit__(None, None, None)          # frees 4 psum banks
        poe_cm = tc.tile_pool(name="poe", bufs=1, space="PSUM")
        attnp["poe"] = poe_cm.__enter__()
        poo_cm = tc.tile_pool(name="poo", bufs=1, space="PSUM")
        attnp["poo"] = poo_cm.__enter__()
        sps_cm = tc.tile_pool(name="sps", bufs=2, space="PSUM")
        attnp["sps"] = sps_cm.__enter__()

        attn_v(0, 0)
        attn_v(0, 1)
        scores(2, 0)
        scores(2, 1)
        attn_v(1, 0)
        attn_v(1, 1)
        scores(3, 0)
        scores(3, 1)
        attn_v(2, 0)
        attn_v(2, 1)
        scores(4, 0)
        scores(4, 1)
        attn_v(3, 0)
        attn_v(3, 1)
        scores(5, 0)
        scores(5, 1)
        attn_v(4, 0)
        attn_v(4, 1)
        scores(6, 0)
        scores(6, 1)
        attn_v(5, 0)
        attn_v(5, 1)
        scores(7, 0)
        scores(7, 1)
        attn_v(6, 0)
        attn_v(6, 1)
        attn_v(7, 0)
        attn_v(7, 1)
        sps_cm.__exit__(None, None, None)
        poo_cm.__exit__(None, None, None)
        poe_cm.__exit__(None, None, None)

        # ---------------- out projection
        prx_cm.__exit__(None, None, None)         # frees proj SBUF

        with tc.tile_pool(name="late", bufs=1) as late, \
             tc.tile_pool(name="ob", bufs=2) as opool, \
             tc.tile_pool(name="ps3", bufs=4, space="PSUM") as ps3p:
            wp = [late.tile([128, C], BF16, tag=f"wp{kc}", name=f"wp{kc}")
                  for kc in range(CT)]
            for kc in range(CT):
                nc.sync.dma_start(wp[kc][:],
                                  wpT_d[128 * kc:128 * (kc + 1), :])
            for tb in range(4):
                ps = [ps3p.tile([128, 512], F32, tag="o3", name="po")
                      for _ in range(4)]
                for ct in range(CT):
                    for x in range(4):
                        tn = 2 * tb + x // 2
                        jc = x % 2
                        nc.tensor.matmul(
                            ps[x][:],
                            oT[ct][:, 128 * tn:128 * (tn + 1)],
                            wp[ct][:, 512 * jc:512 * (jc + 1)],
                            start=(ct == 0), stop=(ct == CT - 1))
                for x in range(4):
                    tn = 2 * tb + x // 2
                    jc = x % 2
                    ob = opool.tile([128, 512], F32, tag="ob")
                    nc.scalar.copy(ob[:], ps[x][:])
                    nc.sync.dma_start(
                        out_d[128 * tn:128 * (tn + 1),
                              512 * jc:512 * (jc + 1)],
                        ob[:])


# ---------------------------------------------------------------- entry
_CACHE = {}


def _get_nc():
    if "nc" not in _CACHE:
        nc = bacc.Bacc("TRN2", target_bir_lowering=False, debug=False,
                       num_devices=B)
        build_kernel(nc)
        nc.compile()
        _CACHE["nc"] = nc
    return _CACHE["nc"]


def make_in_maps(tensor, w_qkv, w_proj):
    tensor = np.asarray(tensor, np.float32)
    wqT, wkT, wvT, wpT = _host_weights(w_qkv, w_proj)
    cosT2, sinP, vconst, consts = _host_tables()
    shared = {"wqT": wqT, "wkT": wkT, "wvT": wvT, "wpT": wpT,
              "cosT2": cosT2, "sinP": sinP, "vconst": vconst,
              "consts": consts}
    bf = ml_dtypes.bfloat16
    in_maps = []
    for i in range(B):
        m = dict(shared)
        m["xT"] = np.ascontiguousarray(tensor[i].T).astype(bf)
        in_maps.append(m)
    return in_maps


def run(tensor, w_qkv, w_proj, trace=False):
    in_maps = make_in_maps(tensor, w_qkv, w_proj)
    nc = _get_nc()
    res = bass_utils.run_bass_kernel_spmd(nc, in_maps, core_ids=list(range(B)),
                                          trace=trace)
    out = np.stack([res.results[i]["out"] for i in range(B)])
    return out, res


def kernel(tensor, w_qkv, w_proj):
    out, _ = run(tensor, w_qkv, w_proj, trace=False)
    return out.astype(np.float32)
